# revision 1
# baseline (speedup 1.0000x reference)
"""Trainium2 Bass kernel for nn_CNNRNN_63625645523427.

Model: relu-gated LSTM decoder over label embeddings with per-step class
prediction.

  proj_img = img @ proj_I; x0 = relu(proj_img); pred0 = x0 @ U_l.T
  per step t:  gates = relu(lab_t @ W_ih.T + b_ih + h @ W_hh.T + b_hh)
               c = f*c + i*cg;  h = o * relu(c)
               x_t = relu(proj_img + h @ proj_O);  pred_t = x_t @ U_l.T

Sharding (8 cores): the recurrence is replicated on every core with the full
batch B=128 (PE matmul cost is independent of M<=128).  The large prediction
matmul [128,512]@[512,7178] is sharded over the class dim C: each core
computes an 898-wide slice of the logits.  Host gathers/concats.

Numerics (validated against a fp64 reference simulation, total rel err
~9e-3 vs the 2e-2 budget):
  - lab @ W_ih runs in fp8e4m3 DoubleRow mode (both operands e4m3): the
    label-path quantization barely perturbs the output (1.5e-3 alone).
  - h @ W_hh stays fp16: the recurrent path amplifies e4m3 noise past the
    error budget (2.2e-2 alone), and hi+lo fp8 splitting needs 4x the
    MACs, cancelling DoubleRow's speedup.
  - h @ proj_O runs in fp8 DoubleRow (8.6e-3 alone; x noise is diluted by
    the exact proj_img term).
  - bias (b_ih+b_hh) is injected into the gates PSUM by a one-hot fp16
    matmul that opens each accumulation group, so the gate relu reads
    PSUM directly and no vector-engine bias add exists.
  - elementwise chain and stored logits are fp16 (adds <1e-4).

Engine budget per step: PE ~8.5us (the wall), ACT ~5.5us (gate relus
from PSUM + xT copies + pred copies), DVE ~4.5us (cell muls + fp8
casts + xpre).

PE emission order per step (keeps the tensor stream dense; fragmenting
it triggers clock throttling): h_mms(t) n-outer | bias+lab-DR(t+1) |
nh-transposes(t) | xT-transposes(t-1) | pred(t-1) | x(t).  The last 5
steps' preds are pended and drained at t=NS to cover the tail's serial
chain.  Warm-up identity transposes keep the PE p-state ramping while
the initial weight DMAs land (split across the sync/ACT/gpsimd queues
in consumption order).
"""

import numpy as np
from contextlib import ExitStack

from ml_dtypes import float8_e4m3fn as npe4m3

import concourse.bass as bass
import concourse.tile as tile
import concourse.mybir as mybir
from concourse import bacc
from concourse.bass_utils import run_bass_kernel_spmd
from concourse.masks import make_identity

B = 128          # batch
T = 20           # labels per sample (output steps)
NS = T - 1       # recurrent steps
L = 512          # feature width
C = 7178         # num classes
G = 4 * L        # gates width
NCORES = 8
CS = 898         # per-core class shard (8*898 = 7184 >= 7178)
KL = L // 128    # K chunks for a 512 contraction

F32 = mybir.dt.float32
F16 = mybir.dt.float16
F8 = mybir.dt.float8e4
NPF16 = np.float16
RELU = mybir.ActivationFunctionType.Relu
DR = mybir.MatmulPerfMode.DoubleRow

_CACHED_NC = None
LAST_RESULT = None


def _build():
    nc = bacc.Bacc("TRN2", target_bir_lowering=False, debug=False,
                   num_devices=NCORES)

    # Inputs pre-arranged on host to [128, k, n] partition-major layouts.
    d_lab8 = nc.dram_tensor("lab8", [NS, 128, KL, B], F8, kind="ExternalInput")
    d_wih8 = nc.dram_tensor("wih8", [128, KL, G], F8, kind="ExternalInput")
    d_whh = nc.dram_tensor("whh", [128, KL, G], F16, kind="ExternalInput")
    d_po8 = nc.dram_tensor("po8", [128, KL, L], F8, kind="ExternalInput")
    d_projI = nc.dram_tensor("projI", [128, KL, L], F16, kind="ExternalInput")
    d_imgT = nc.dram_tensor("imgT", [128, KL, B], F16, kind="ExternalInput")
    d_ulT = nc.dram_tensor("ulT", [128, KL, CS], F16, kind="ExternalInput")
    d_h0T = nc.dram_tensor("h0T", [128, KL, B], F16, kind="ExternalInput")
    d_bias = nc.dram_tensor("biasr", [1, G], F16, kind="ExternalInput")
    d_c0 = nc.dram_tensor("c0b", [B, L], F16, kind="ExternalInput")
    d_out = nc.dram_tensor("preds", [T, B, CS], F16, kind="ExternalOutput")

    with tile.TileContext(nc) as tc, ExitStack() as ctx:
        consts = ctx.enter_context(tc.tile_pool(name="consts", bufs=1))
        labp = ctx.enter_context(tc.tile_pool(name="lab", bufs=3))
        act = ctx.enter_context(tc.tile_pool(name="act", bufs=3))
        gatep = ctx.enter_context(tc.tile_pool(name="gate", bufs=3))
        predp = ctx.enter_context(tc.tile_pool(name="pred", bufs=3))
        psum = ctx.enter_context(tc.tile_pool(name="ps", bufs=6, space="PSUM"))
        psum_p = ctx.enter_context(
            tc.tile_pool(name="psp", bufs=2, space="PSUM"))

        # --- constants / weights -------------------------------------------
        # Three DMA queues (sync, scalar/ACT, gpsimd), each ~1.6MB, ordered
        # by first consumption: phase-0 inputs and gates(1) operands land
        # first; whh is split by gate column (n) matching h_mms' n-outer
        # consumption so h_mms(1) can start before whh fully lands.
        ident = consts.tile([128, 128], F16, tag="ident")
        make_identity(nc, ident[:])
        bias_sb = consts.tile([128, G], F16, tag="bias")
        nc.vector.memset(bias_sb[:], 0.0)
        nc.scalar.dma_start(bias_sb[0:1, :], d_bias.ap())
        onehot = consts.tile([128, 128], F16, tag="onehot")
        nc.vector.memset(onehot[:], 0.0)
        nc.vector.memset(onehot[0:1, :], 1.0)
        imgT_sb = consts.tile([128, KL, B], F16, tag="imgT")
        nc.scalar.dma_start(imgT_sb[:], d_imgT.ap())
        projI_sb = consts.tile([128, KL, L], F16, tag="projI")
        nc.sync.dma_start(projI_sb[:], d_projI.ap())
        lab1_sb = labp.tile([128, KL, B], F8, tag="lab")
        nc.sync.dma_start(lab1_sb[:], d_lab8.ap()[0])
        hT = act.tile([128, KL, B], F16, tag="hT")
        nc.sync.dma_start(hT[:], d_h0T.ap())
        ulT_sb = consts.tile([128, KL, CS], F16, tag="ulT")
        nc.gpsimd.dma_start(ulT_sb[:, :, 0:512], d_ulT.ap()[:, :, 0:512])
        nc.gpsimd.dma_start(ulT_sb[:, :, 512:CS], d_ulT.ap()[:, :, 512:CS])
        wih8_sb = consts.tile([128, KL, G], F8, tag="wih8")
        nc.scalar.dma_start(wih8_sb[:], d_wih8.ap())
        whh_sb = consts.tile([128, KL, G], F16, tag="whh")
        for n, q in zip(range(4), (nc.sync, nc.gpsimd, nc.scalar, nc.gpsimd)):
            nsl = slice(512 * n, 512 * (n + 1))
            q.dma_start(whh_sb[:, :, nsl], d_whh.ap()[:, :, nsl])
        po8_sb = consts.tile([128, KL, L], F8, tag="po8")
        nc.sync.dma_start(po8_sb[:], d_po8.ap())
        c_prev = act.tile([128, L], F16, tag="c")
        nc.scalar.dma_start(c_prev[:], d_c0.ap())

        def transpose_to(src_sb, tag, dtype=F16, extra8=None):
            """[128, L] fp16 -> [128, KL, 128] via PE transpose, chunked
            PSUM->SBUF copies.  extra8: also emit fp8 cast copies."""
            tp = psum.tile([128, KL, 128], F16, tag="ps")
            for k in range(KL):
                nc.tensor.transpose(
                    tp[:, k, :], src_sb[:, 128 * k:128 * (k + 1)], ident[:])
            dst = act.tile([128, KL, B], dtype, tag=tag)
            dst8 = None
            if extra8 is not None:
                dst8 = act.tile([128, KL, B], F8, tag=extra8)
            for k in range(KL):
                nc.vector.tensor_copy(dst[:, k, :], tp[:, k, :])
                if dst8 is not None:
                    nc.vector.tensor_copy(dst8[:, k, :], tp[:, k, :])
            return dst, dst8

        def transpose_to_act(src_sb, tag):
            """Same but copies on the scalar engine (for xT; DVE is busier).
            bufs=7: up to 5 pended xT tiles stay live at the tail."""
            tp = psum.tile([128, KL, 128], F16, tag="ps")
            for k in range(KL):
                nc.tensor.transpose(
                    tp[:, k, :], src_sb[:, 128 * k:128 * (k + 1)], ident[:])
            dst = act.tile([128, KL, B], F16, tag=tag, bufs=7)
            for k in range(KL):
                nc.scalar.copy(dst[:, k, :], tp[:, k, :])
            return dst

        def lab_block(tiles, lab_sb, stop):
            for n in range(4):
                nsl = slice(512 * n, 512 * (n + 1))
                for p in range(2):
                    sl = slice(2 * p, 2 * p + 2)
                    nc.tensor.matmul(tiles[n][:], lab_sb[:, sl, :],
                                     wih8_sb[:, sl, nsl],
                                     start=False, stop=(stop and p == 1),
                                     perf_mode=DR)

        def open_gates(t, lab_sb=None):
            """Open 4 gates psum tiles: bias one-hot MM + lab fp8-DR MMs.
            Emitted right after h_mms(t) so this work fills the PE while
            the elemwise chain runs; tile n's slot only waits on the
            previous step's relu of the same gate (done early, n-outer)."""
            if lab_sb is None:
                lab_sb = labp.tile([128, KL, B], F8, tag="lab")
                nc.sync.dma_start(lab_sb[:], d_lab8.ap()[t - 1])
            tiles = [psum.tile([128, 512], F32, tag="ps", name=f"gps{n}")
                     for n in range(4)]
            for n in range(4):
                nc.tensor.matmul(tiles[n][:], onehot[:],
                                 bias_sb[:, 512 * n:512 * (n + 1)],
                                 start=True, stop=False)
            lab_block(tiles, lab_sb, stop=False)
            return tiles, lab_sb

        def h_mms(gtiles, hT_, stop=True):
            """Accumulate the fp16 h-part; n-outer so gate tile n closes
            (and its relu can start) before tile n+1's matmuls run."""
            for n in range(4):
                nsl = slice(512 * n, 512 * (n + 1))
                for k in range(KL):
                    nc.tensor.matmul(gtiles[n][:], hT_[:, k, :],
                                     whh_sb[:, k, nsl],
                                     start=False, stop=(stop and k == KL - 1))

        def gates_elemwise(gtiles, c_prev):
            """relu from PSUM (bias already accumulated); fp16 cell math.
            Gate tiles close in order i,f,cg,o (n-outer h_mms); the o-gate
            relu and nh mul are chunked so the nh transposes (the critical
            path into the next step's h matmuls) start on the first chunk
            instead of waiting for the full 512-wide chain."""
            gi = gatep.tile([128, 512], F16, tag="grelu0")
            nc.scalar.activation(gi[:], gtiles[0][:], RELU)
            gf = gatep.tile([128, 512], F16, tag="grelu1")
            nc.vector.tensor_scalar_max(gf[:], gtiles[1][:], 0.0)
            gcg = gatep.tile([128, 512], F16, tag="grelu2")
            nc.scalar.activation(gcg[:], gtiles[2][:], RELU)
            t1 = act.tile([128, L], F16, tag="t1")
            nc.vector.tensor_mul(t1[:], gf[:], c_prev[:])
            t2 = act.tile([128, L], F16, tag="t2")
            nc.vector.tensor_mul(t2[:], gi[:], gcg[:])
            c_new = act.tile([128, L], F16, tag="c")
            nc.vector.tensor_add(c_new[:], t1[:], t2[:])
            rc = act.tile([128, L], F16, tag="rc")
            nc.scalar.activation(rc[:], c_new[:], RELU)
            go = gatep.tile([128, 512], F16, tag="grelu3")
            nc.vector.tensor_scalar_max(go[:], gtiles[3][:], 0.0)
            nh = act.tile([128, L], F16, tag="nh")
            nc.vector.tensor_mul(nh[:], go[:], rc[:])
            return nh, c_new

        def pred_lo(xT_sb):
            ps1 = psum_p.tile([128, 512], F32, tag="psp")
            for k in range(KL):
                nc.tensor.matmul(ps1[:], xT_sb[:, k, :], ulT_sb[:, k, 0:512],
                                 start=(k == 0), stop=(k == KL - 1))
            return ps1

        def pred_hi_and_store(ps1, xT_sb, t):
            ps2 = psum_p.tile([128, CS - 512], F32, tag="psp")
            for k in range(KL):
                nc.tensor.matmul(ps2[:], xT_sb[:, k, :], ulT_sb[:, k, 512:CS],
                                 start=(k == 0), stop=(k == KL - 1))
            pred_sb = predp.tile([128, CS], F16, tag="pred")
            nc.scalar.copy(pred_sb[:, 0:512], ps1[:])
            nc.scalar.copy(pred_sb[:, 512:CS], ps2[:])
            nc.sync.dma_start(d_out.ap()[t], pred_sb[:])

        def x_step(h8_):
            """x = relu(proj_img + h8 @ po8), fp8 DoubleRow."""
            xps = psum_p.tile([128, L], F32, tag="psp")
            for p in range(2):
                sl = slice(2 * p, 2 * p + 2)
                nc.tensor.matmul(xps[:], h8_[:, sl, :], po8_sb[:, sl, :],
                                 start=(p == 0), stop=(p == 1), perf_mode=DR)
            xpre = act.tile([128, L], F32, tag="xpre")
            nc.vector.tensor_add(xpre[:], xps[:], proj_img[:])
            x_sb = act.tile([128, L], F16, tag="x")
            nc.scalar.activation(x_sb[:], xpre[:], RELU)
            return x_sb

        # warm-up A: stream the PE on the identity while the first DMAs
        # land (the p-state ramps toward full clock with continuous work)
        warma = psum.tile([128, KL, 128], F16, tag="ps")
        for i in range(24):
            nc.tensor.transpose(warma[:, i % KL, :], ident[:], ident[:])

        # --- phase 0: proj_img, x0 = relu(proj_img), pred0 -----------------
        pi_ps = psum.tile([128, L], F32, tag="ps")
        for k in range(KL):
            nc.tensor.matmul(pi_ps[:], imgT_sb[:, k, :], projI_sb[:, k, :],
                             start=(k == 0), stop=(k == KL - 1))
        proj_img = consts.tile([128, L], F32, tag="projimg")
        nc.vector.tensor_copy(proj_img[:], pi_ps[:])
        x_prev = act.tile([128, L], F16, tag="x")
        nc.scalar.activation(x_prev[:], pi_ps[:], RELU)

        xT = transpose_to_act(x_prev, "xT")
        ps1 = pred_lo(xT)
        pred_hi_and_store(ps1, xT, 0)

        # warm-up: keep the PE streaming (p-state ramps toward 2.4 GHz
        # only after ~3us of continuous work) while the remaining weights
        # DMA in; harmless transposes of the identity into a scratch tile.
        warm = psum.tile([128, KL, 128], F16, tag="ps")
        for i in range(40):
            nc.tensor.transpose(warm[:, i % KL, :], ident[:], ident[:])

        # --- software-pipelined main loop ----------------------------------
        # PE emission order per step: h_mms(t) | bias+lab(t+1) |
        # nh-transposes(t) | xT-transposes(t-1) | pred(t-1) | x(t).
        # open_gates has no dependency on this step's elemwise (its psum
        # slots wait only on last step's early relus), so it fills the PE
        # while ACT/DVE produce nh; the xT transposes sit late because
        # x(t-1) relu lands mid-step on the ACT queue.
        # step-1 gates in the order bias -> h -> lab: the h weights' first
        # column block lands before the (later-queued) lab weights, so the
        # PE starts real work sooner after phase 0.
        gtiles = [psum.tile([128, 512], F32, tag="ps", name=f"g1ps{n}")
                  for n in range(4)]
        for n in range(4):
            nc.tensor.matmul(gtiles[n][:], onehot[:],
                             bias_sb[:, 512 * n:512 * (n + 1)],
                             start=True, stop=False)
        h_mms(gtiles, hT, stop=False)
        lab_block(gtiles, lab1_sb, stop=True)

        pend = []
        for t in range(1, NS + 1):
            if t > 1:
                h_mms(gtiles, hT)
            cur_gtiles = gtiles
            if t < NS:
                gtiles, _ = open_gates(t + 1)
            else:
                # drain pended preds here: they fill the PE through the
                # final step's elemwise chain
                for pxT, pt in pend:
                    ps1 = pred_lo(pxT)
                    pred_hi_and_store(ps1, pxT, pt)
                pend = []
            nh, c_prev = gates_elemwise(cur_gtiles, c_prev)
            hT_new, h8_new = transpose_to(nh, "hT", extra8="h8")
            if t > 1:
                xT = transpose_to_act(x_prev, "xT")
                if t >= NS - 4:
                    pend.append((xT, t - 1))
                else:
                    ps1 = pred_lo(xT)
                    pred_hi_and_store(ps1, xT, t - 1)
            x_prev = x_step(h8_new)
            hT = hT_new

        # tail: pend'ed preds cover the last x chain.
        (pxT, pt), = pend
        ps1p = pred_lo(pxT)
        xT = transpose_to_act(x_prev, "xT")
        pred_hi_and_store(ps1p, pxT, pt)
        ps1 = pred_lo(xT)
        pred_hi_and_store(ps1, xT, NS)

    nc.compile()
    return nc


def kernel(img_embeddings, labels_idx, U_l, proj_I, proj_O,
           W_ih, b_ih, W_hh, b_hh, h0, c0):
    global _CACHED_NC, LAST_RESULT
    img = np.asarray(img_embeddings, np.float32)
    idx = np.asarray(labels_idx)
    U_l = np.asarray(U_l, np.float32)
    proj_I = np.asarray(proj_I, np.float32)
    proj_O = np.asarray(proj_O, np.float32)
    W_ih = np.asarray(W_ih, np.float32)
    W_hh = np.asarray(W_hh, np.float32)
    b_ih = np.asarray(b_ih, np.float32)
    b_hh = np.asarray(b_hh, np.float32)
    h0 = np.asarray(h0, np.float32)
    c0 = np.asarray(c0, np.float32)

    def bf(x):
        return np.ascontiguousarray(x.astype(NPF16))

    def b8(x):
        return np.ascontiguousarray(
            np.clip(x, -240.0, 240.0).astype(npe4m3))

    def pkn(x):
        # [k*128, n] -> [128, k, n] partition-major for contiguous DMA
        kk = x.shape[0] // 128
        return np.ascontiguousarray(
            x.reshape(kk, 128, x.shape[1]).transpose(1, 0, 2))

    lab = U_l[idx[:, :NS]]                                   # [B, NS, L]
    labT = lab.transpose(1, 2, 0)                            # [NS, L, B]
    lab8 = b8(np.stack([pkn(labT[t]) for t in range(NS)]))   # [NS,128,KL,B]
    wih8 = b8(pkn(W_ih.T))
    whh = bf(pkn(W_hh.T))
    po8 = b8(pkn(proj_O))
    biasr = bf((b_ih + b_hh)[None, :])                       # [1, G]
    imgT = bf(pkn(img.T))
    h0b = pkn(np.broadcast_to(h0[:, None], (L, B)))
    c0b = bf(np.broadcast_to(c0[None, :], (B, L)))
    ulT = np.zeros((L, NCORES * CS), np.float32)
    ulT[:, :C] = U_l.T

    if _CACHED_NC is None:
        _CACHED_NC = _build()
    nc = _CACHED_NC

    common = {
        "lab8": lab8, "wih8": wih8, "whh": whh, "po8": po8,
        "projI": bf(pkn(proj_I)), "imgT": imgT,
        "h0T": bf(h0b), "biasr": biasr, "c0b": c0b,
    }
    in_maps = [
        dict(common, ulT=bf(pkn(ulT[:, c * CS:(c + 1) * CS])))
        for c in range(NCORES)
    ]

    res = run_bass_kernel_spmd(nc, in_maps, core_ids=list(range(NCORES)))
    LAST_RESULT = res
    if res.exec_time_ns is not None:
        print(f"HW exec time: {res.exec_time_ns} ns")

    allpred = np.concatenate(
        [res.results[c]["preds"].astype(np.float32) for c in range(NCORES)],
        axis=2)
    out = np.ascontiguousarray(allpred[:, :, :C].transpose(1, 0, 2))
    return out

